# revision 2
# baseline (speedup 1.0000x reference)
"""LE-ACE forward kernel for 8 Trainium2 NeuronCores.

Design (per core, 750 atoms of 6000, feature-major layout):
  - Atoms sorted by (structure,species) segment on host, sharded contiguously.
  - sph_l shipped as [Q_l*(2l+1), 768] column slices (atoms in free dim).
  - nu=2 B2: per diag (l,l) tuple, u = one-hot gather matmul of sph rows,
    w = cg/mult-weighted mix matmul; B2 = sum_m1 u_m1 * w_m1 on DVE.
  - nu=3 B3: gather indices composed on host (i1[iA[k']]); operand tiles are
    built by one matmul per *pair* of CG a-indices (two 64-row halves stacked
    into 128 partitions), products on DVE, pair-sum via a [I;I] fold matmul.
  - F [960, 768] per core DMA'd out; segment-sum + assembly done on host.
"""

import numpy as np
import sys

sys.path.insert(0, "/opt/trn_rl_repo")

import concourse.bass as bass
import concourse.mybir as mybir
from concourse.bass_utils import run_bass_kernel_spmd
from concourse.tile import TileContext

# --- fixed LE-ACE configuration (matches the problem spec) ---
L2 = [(0, 0), (0, 1), (0, 2), (0, 3), (1, 1), (1, 2), (1, 3), (2, 2), (2, 3), (3, 3)]
DIAG2 = [0, 4, 7, 9]
L3 = [(0, 0, 0), (0, 1, 1), (0, 2, 2), (0, 3, 3), (1, 1, 2), (1, 2, 3), (2, 2, 2)]
PARENT3 = [0, 1, 2, 3, 4, 5, 7]
Q = [32, 24, 16, 8]
N_SPECIES = 4
K2 = 128
K3 = 64
N_ATOMS = 6000
N_STRUCT = 48
N_CORES = 8
PER = N_ATOMS // N_CORES          # 750 atoms per core
NPAD = 768                        # padded per-core atom count
CH = 384                          # free-dim chunk (psum-friendly)
NCHUNK = NPAD // CH
ROWS = [Q[l] * (2 * l + 1) for l in range(4)]   # 32, 72, 80, 56
NB2 = 4 * K2                      # 512 B2 feature rows
NB3 = 7 * K3                      # 448 B3 feature rows
F32 = mybir.dt.float32

MAXW = 1  # this container's walrus accepts few sync waits per instruction


def _pairs_for(t3):
    l1, l2, _ = L3[t3]
    mA = (2 * l1 + 1) * (2 * l2 + 1)
    a = list(range(mA))
    return [a[i:i + 2] for i in range(0, mA, 2)]


def build_consts(mult2, cg2, mult3, cg3, idx2, idx3):
    """Host-side constant lhsT matrices for all gather/mix matmuls (f32)."""
    c = {}
    for j, t in enumerate(DIAG2):
        l = L2[t][0]
        d = 2 * l + 1
        i1 = np.asarray(idx2[t][0]); i2 = np.asarray(idx2[t][1])
        m2v = np.asarray(mult2[t], np.float32)
        cg = np.asarray(cg2[j], np.float32)[0]          # [d*d]
        U = np.zeros((ROWS[l], d * K2), np.float32)
        W = np.zeros((ROWS[l], d * K2), np.float32)
        for m1 in range(d):
            U[i1 * d + m1, m1 * K2 + np.arange(K2)] = 1.0
            for m2 in range(d):
                W[i2 * d + m2, m1 * K2 + np.arange(K2)] = m2v * cg[m1 * d + m2]
        c[f"b2u{j}"] = U
        c[f"b2w{j}"] = W
    for t3, (l1, l2, l3) in enumerate(L3):
        p = PARENT3[t3]
        i1p = np.asarray(idx2[p][0]); i2p = np.asarray(idx2[p][1])
        mp = np.asarray(mult2[p], np.float32)
        iA = np.asarray(idx3[t3][0]); iS = np.asarray(idx3[t3][1])
        m3 = np.asarray(mult3[t3], np.float32)
        cg = np.asarray(cg3[t3], np.float32)[0]         # [mA*(2l3+1)]
        d1, d2, d3 = 2 * l1 + 1, 2 * l2 + 1, 2 * l3 + 1
        pairs = _pairs_for(t3)
        P = len(pairs)
        Ut = np.zeros((ROWS[l1], P * 128), np.float32)
        Vt = np.zeros((ROWS[l2], P * 128), np.float32)
        Wt = np.zeros((ROWS[l3], P * 128), np.float32)
        ks = np.arange(K3)
        for pi, pr in enumerate(pairs):
            for half, a in enumerate(pr):
                m1, m2 = divmod(a, d2)
                col = pi * 128 + half * 64 + ks
                Ut[i1p[iA] * d1 + m1, col] = mp[iA]
                Vt[i2p[iA] * d2 + m2, col] = 1.0
                for cc in range(d3):
                    Wt[iS * d3 + cc, col] = m3 * cg[a * d3 + cc]
        c[f"c3u{t3}"] = Ut
        c[f"c3v{t3}"] = Vt
        c[f"c3w{t3}"] = Wt
    fold = np.zeros((128, 64), np.float32)
    fold[np.arange(64), np.arange(64)] = 1.0
    fold[np.arange(64) + 64, np.arange(64)] = 1.0
    c["fold"] = fold
    return c


def split_waits(nc, maxw=MAXW):
    """Split >maxw semaphore waits off onto same-engine NoOps (walrus limit)."""
    for bb in nc.m.functions[0].blocks:
        new_insts = []
        for ins in bb.instructions:
            si = ins.sync_info
            if si is not None and si.on_wait and len(si.on_wait) > maxw:
                waits = list(si.on_wait)
                extra, keep = waits[:-maxw], waits[-maxw:]
                k = 0
                while extra:
                    chunk, extra = extra[:maxw], extra[maxw:]
                    new_insts.append(mybir.InstNoOp(
                        name=f"{ins.name}-wsplit{k}",
                        engine=ins.engine,
                        sync_info=mybir.SyncInfo(on_wait=chunk, on_update=[]),
                        bass_nofuse=True,
                    ))
                    k += 1
                ins.sync_info = mybir.SyncInfo(
                    on_wait=keep, on_update=list(si.on_update))
            new_insts.append(ins)
        bb.instructions = new_insts
    return nc


def build_program(const_shapes):
    nc = bass.Bass()
    sph_in = [nc.declare_dram_parameter(f"sph{l}f", [ROWS[l], NPAD], F32,
                                        isOutput=False) for l in range(4)]
    cin = {name: nc.declare_dram_parameter(name, list(shape), F32, isOutput=False)
           for name, shape in const_shapes.items()}
    fout = nc.declare_dram_parameter("fout", [NB2 + NB3, NPAD], F32, isOutput=True)

    with TileContext(nc) as tc:
        with tc.tile_pool(name="const", bufs=1) as cp, \
             tc.tile_pool(name="sph", bufs=2) as sp, \
             tc.tile_pool(name="work", bufs=10) as wp, \
             tc.tile_pool(name="acc", bufs=4) as ap, \
             tc.tile_pool(name="ps", bufs=2, space="PSUM") as ps:

            ct = {}
            for name, dram in cin.items():
                t = cp.tile(list(dram.shape), F32, tag=f"c_{name}")
                nc.gpsimd.dma_start(t[:], dram[:])
                ct[name] = t

            for j in range(NCHUNK):
                n0 = j * CH
                sph = []
                for l in range(4):
                    t = sp.tile([ROWS[l], CH], F32, tag=f"sph{l}")
                    nc.gpsimd.dma_start(t[:], sph_in[l][:, n0:n0 + CH])
                    sph.append(t)

                # ---- B2 (nu=2, diagonal tuples) ----
                for jj, t2 in enumerate(DIAG2):
                    l = L2[t2][0]
                    d = 2 * l + 1
                    acc2 = ap.tile([128, CH], F32, tag="acc2")
                    for m1 in range(d):
                        up = ps.tile([128, CH], F32, tag="up")
                        nc.tensor.matmul(up[:], ct[f"b2u{jj}"][:, m1 * K2:(m1 + 1) * K2],
                                         sph[l][:], start=True, stop=True)
                        us = wp.tile([128, CH], F32, tag="us")
                        nc.scalar.copy(us[:], up[:])
                        wps = ps.tile([128, CH], F32, tag="wp")
                        nc.tensor.matmul(wps[:], ct[f"b2w{jj}"][:, m1 * K2:(m1 + 1) * K2],
                                         sph[l][:], start=True, stop=True)
                        ws = wp.tile([128, CH], F32, tag="ws")
                        nc.scalar.copy(ws[:], wps[:])
                        if m1 == 0:
                            nc.vector.tensor_mul(acc2[:], us[:], ws[:])
                        else:
                            tm = wp.tile([128, CH], F32, tag="tm")
                            nc.vector.tensor_mul(tm[:], us[:], ws[:])
                            nc.vector.tensor_add(acc2[:], acc2[:], tm[:])
                    nc.gpsimd.dma_start(fout[jj * K2:(jj + 1) * K2, n0:n0 + CH], acc2[:])

                # ---- B3 (nu=3 tuples) ----
                for t3, (l1, l2, l3) in enumerate(L3):
                    pairs = _pairs_for(t3)
                    acc3 = ap.tile([128, CH], F32, tag="acc3")
                    for pi in range(len(pairs)):
                        cs = slice(pi * 128, (pi + 1) * 128)
                        up = ps.tile([128, CH], F32, tag="up")
                        nc.tensor.matmul(up[:], ct[f"c3u{t3}"][:, cs], sph[l1][:],
                                         start=True, stop=True)
                        us = wp.tile([128, CH], F32, tag="us")
                        nc.scalar.copy(us[:], up[:])
                        vp = ps.tile([128, CH], F32, tag="wp")
                        nc.tensor.matmul(vp[:], ct[f"c3v{t3}"][:, cs], sph[l2][:],
                                         start=True, stop=True)
                        vs = wp.tile([128, CH], F32, tag="ws")
                        nc.scalar.copy(vs[:], vp[:])
                        wp3 = ps.tile([128, CH], F32, tag="wp3")
                        nc.tensor.matmul(wp3[:], ct[f"c3w{t3}"][:, cs], sph[l3][:],
                                         start=True, stop=True)
                        ws3 = wp.tile([128, CH], F32, tag="ws3")
                        nc.scalar.copy(ws3[:], wp3[:])
                        tm = wp.tile([128, CH], F32, tag="tm")
                        nc.vector.tensor_mul(tm[:], us[:], vs[:])
                        if pi == 0:
                            nc.vector.tensor_mul(acc3[:], tm[:], ws3[:])
                        else:
                            tm2 = wp.tile([128, CH], F32, tag="tm2")
                            nc.vector.tensor_mul(tm2[:], tm[:], ws3[:])
                            nc.vector.tensor_add(acc3[:], acc3[:], tm2[:])
                    fp = ps.tile([64, CH], F32, tag="fp")
                    nc.tensor.matmul(fp[:], ct["fold"][:], acc3[:], start=True, stop=True)
                    fs = wp.tile([64, CH], F32, tag="fs")
                    nc.scalar.copy(fs[:], fp[:])
                    nc.gpsimd.dma_start(
                        fout[NB2 + t3 * K3: NB2 + (t3 + 1) * K3, n0:n0 + CH], fs[:])
    return split_waits(nc)


_PROG_CACHE = {}


def kernel(sph0, sph1, sph2, sph3, radial_spectrum, composition_features,
           mult2, cg2, mult3, cg3, idx2, idx3, atom_idx, n_structures):
    sphs = [np.asarray(s, np.float32) for s in (sph0, sph1, sph2, sph3)]
    rad = np.asarray(radial_spectrum, np.float32)
    comp = np.asarray(composition_features, np.float32)
    aidx = np.asarray(atom_idx)
    S = int(n_structures)
    nseg = S * N_SPECIES

    consts = build_consts(mult2, cg2, mult3, cg3, idx2, idx3)

    perm = np.argsort(aidx, kind="stable")
    seg_sorted = aidx[perm]

    key = "prog"
    if key not in _PROG_CACHE:
        _PROG_CACHE[key] = build_program({k: v.shape for k, v in consts.items()})
    nc = _PROG_CACHE[key]

    in_maps = []
    for cidx in range(N_CORES):
        sel = perm[cidx * PER:(cidx + 1) * PER]
        m = dict(consts)
        for l in range(4):
            flat = sphs[l].reshape(ROWS[l], N_ATOMS)
            sl = np.zeros((ROWS[l], NPAD), np.float32)
            sl[:, :PER] = flat[:, sel]
            m[f"sph{l}f"] = sl
        in_maps.append(m)

    res = run_bass_kernel_spmd(nc, in_maps, list(range(N_CORES)))
    F = np.concatenate([np.asarray(res.results[i]["fout"])[:, :PER]
                        for i in range(N_CORES)], axis=1)   # [960, 6000] perm order

    onehot = np.zeros((nseg, N_ATOMS), np.float32)
    onehot[seg_sorted, np.arange(N_ATOMS)] = 1.0
    P23 = onehot @ F.T                                      # [192, 960]
    cr = np.concatenate([comp, rad], axis=1)[perm]          # [6000, 65]
    P01 = onehot @ cr                                       # [192, 65]

    b0 = P01[:, :1].reshape(S, N_SPECIES).sum(axis=1).reshape(S, 1)
    b1 = P01[:, 1:].reshape(S, -1)
    b2 = P23[:, :NB2].reshape(S, -1)
    b3 = P23[:, NB2:].reshape(S, -1)
    return np.concatenate([b0, b1, b2, b3], axis=1).astype(np.float32)
